# revision 84
# baseline (speedup 1.0000x reference)
"""RBF-kernel SVM prediction on 8 Trainium2 NeuronCores.

predictions = exp(-g*||x_i - t_j||^2) @ (alphas*y) + b,  g = 0.5

Strategy (per sharding hint): shard X rows 8-way, replicate train side.
Math is factorized as
    pred_i = exp(-g*||x_i||^2) * sum_j y_j * exp(x_i . t_j + c_j) + b
    c_j    = -g*||t_j||^2 + ln(alpha_j)
so the train-side affine terms ride per-partition biases and the
query-side factor is a per-row epilogue scale. Train points are host-sorted
by label so the +/- y_j signs become whole-tile add/sub.

Per-core engine balance (the loop is exp-throughput bound):
 - PE runs the G = Xt.X^T GEMM in fp8 DoubleRow (2x rate), plus weighted
   ones-matvecs that accumulate ~half the exp'd tiles straight into a
   persistent PSUM row (one accumulation group across the whole loop).
 - ACT does true exp for ~2/3 of tiles (PSUM source, per-partition bias).
 - DVE runs a custom fused op for the rest, computing bf16(e^x) bit
   patterns directly (Schraudolph: bits16 = clamp(A*(G+c)+B, 0, 32000)
   as uint16; the fp32 clamp precedes conversion so no NaN/Inf patterns
   can appear), plus bf16 tensor-tensor accumulates.
GPSIMD is left idle: it shares an SBUF port with the DVE and measured
as dragging all DVE 2x ops down to 1x when used for accumulates.

Scheduling notes (all trace-verified on HW):
 - Accumulator folds are emitted INLINE a few tiles after the
   accumulator's last contribution, and the PSUM chain stops on the last
   matvec, so the post-loop tail is just exp+matvec+p_row+out (~1.7us,
   was ~11us when folds trailed the loop).
 - Startup input triggers split across the sync and scalar HWDGE queues
   (scalar triggers run on the then-idle ACT engine), ordered by first
   use; p_row and the output DMA go out in halves so half 0 overlaps
   half 1's folds.
 - With 3 PSUM g-buffers, GEMM(t+3) waits on exp(t): anything that
   delays an exp behind it on the DVE queue stalls the PE. Measured
   dead ends (all REGRESSED, do not retry): CCE accumulate-DMA groups
   (~215GB/s effective, 8-11us per 1.5-2MB, serialized per-dest;
   +10..25us), GPSIMD tensor_tensor accumulates (2.1-2.9us/tile and its
   SWDGE triggers block the queue), moving mv tiles to DVE TT (+5us for
   4 tiles — DVE queue jitter), paired [128,2048] TTs via shared pair
   buffers (system-wide SBUF contention, +15us), single [1,1024]-out
   matvecs (ISA rejects PSUM bank crossing).
"""

import os
import sys

import numpy as np

for _p in ("/opt/trn_rl_repo", "/root/.axon_site/_ro/trn_rl_repo"):
    if os.path.isdir(_p) and _p not in sys.path:
        sys.path.append(_p)

import ml_dtypes

import concourse.bass as bass
import concourse.tile as tile
from concourse import bacc, mybir
from concourse.bass_utils import run_bass_kernel_spmd

GAMMA = 0.5
N, M, D = 8192, 8192, 256
NCORES = 8
IC = N // NCORES          # query rows per core (1024)
JT = M // 128             # j-tiles (64)
F32 = mybir.dt.float32
BF16 = mybir.dt.bfloat16
FP8 = mybir.dt.float8e4
U16 = mybir.dt.uint16
FP32_MIN_NORMAL = 1.1754944e-38

# Schraudolph constants for bf16: bits16(e^x) ~ A*x + B, clamped to [0, CLAMP_HI].
SCH_A = 184.6650784   # 128 / ln(2)
SCH_B = 16250.496     # 128 * (127 - 0.0430)
SCH_CLAMP = 32000.0   # < 0x7F80 (inf); e^x here never exceeds e^10 anyway

def _env(name, default):
    return int(os.environ.get(name, default))


N_MAGIC = _env("KN_MAGIC", 21)    # tiles exp'd on the DVE
N_DMA_GRP = _env("KN_DMAGRP", 0)  # groups of tiles accumulated by CCE DMAs
GRP_SZ = _env("KN_GRPSZ", 6)      # tiles per CCE-DMA accumulate group
GRP_MAX = _env("KN_GRPMAX", 40)   # group member tiles must be < this
N_GTT = _env("KN_GTT", 0)         # tiles accumulated by GPSIMD tensor-tensor
N_WARM = _env("KN_WARM", 10)      # PE warmup matmuls (HAM clock-gate release)
MV_TAIL = _env("KN_MVTAIL", 60)   # tiles >= this always accumulate via PE matvec
ACC_LAG = _env("KN_LAG", 2)       # emit tile t's acc after tile t+ACC_LAG's GEMM
EPOOL = _env("KN_EPOOL", 6)       # e-tile pool buffers
NCHUNK_C = _env("KN_NCHUNK", 16)  # xt DMA chunks (must divide 64 tile count)
MAGIC_MV = _env("KN_MAGICMV", 0)  # DVE-exp'd tiles accumulate on the PE
MAGIC_PAIR = _env("KN_MAGICPAIR", 0)  # place DVE-exp tiles in adjacent pairs
MV_PERIOD = _env("KN_MVP", 2)     # tiles t%P==0 accumulate via PE matvec
N_DEMOTE = _env("KN_DEMOTE", 0)   # mv tiles demoted to DVE TT (PE relief)
MV_ONE = _env("KN_MV1", 0)        # single [1,1024] matvec per mv tile
PAIR_TT = _env("KN_PAIR", 0)      # pair adjacent dve tiles: one TT per pair
SWI = _env("KN_SWI", 0)           # DoubleRowSwInterleave weights (fast LDW)
CK_SPLIT = _env("KN_CKSPLIT", 0)  # odd early xt chunks trigger on scalar queue
MV_DR = _env("KN_MVDR", 0)        # fp8 e-tile pairs: one DoubleRow matvec per 2 mv tiles
LAST_M = 0.0                      # global exponent shift (for test.py s-check)

# Set by test harness to collect a profile; harness grading leaves it off.
TRACE = False
DEBUG_S = os.environ.get("KN_DEBUG_S", "") != ""  # emit pre-mask s for tests
LAST_RESULTS = None

_EXP_OP = None


def _register_exp_op():
    """Register the custom DVE op computing bf16 exp bit patterns.

    body = min(max(Src0*C0 + C1, 0), C2), written to a uint16 tile whose
    bits, reinterpreted as bf16, approximate e^(Src0 + c) (C1 carries the
    per-partition c bias pre-scaled by A).
    """
    global _EXP_OP
    if _EXP_OP is not None:
        return _EXP_OP
    from concourse import dve_ops as dvo
    from concourse.dve_ops import DveOp
    from concourse.dve_spec import Spec, Src0, C0, C1, C2, Zero, maxx, minn, lower
    from concourse.dve_uop import DveOpSpec

    name = "EXP_BITS_U16_ANT"
    for op in dvo.OPS:
        if op.name == name:
            _EXP_OP = op
            return op
    spec = Spec(
        body=minn(maxx(Src0 * C0 + C1, Zero), C2),
        reference=lambda in0, in1, s0, s1, imm2: np.minimum(
            np.maximum(in0.astype(np.float32) * s0 + s1, 0.0), imm2
        ),
    )
    opcode = dvo._CUSTOM_DVE_ROW_BASE + len(dvo.OPS)
    shas = {}
    for ver in ("v3", "v4"):
        tmp = DveOpSpec(name=name, opcode=opcode, uops=lower(spec, ver=ver),
                        rd1_en=False)
        shas[ver] = tmp.sha(ver)
    op = DveOp(name, spec, subdim=False, uops_sha=shas)
    dvo.OPS.append(op)
    dvo.CUSTOM_DVE_SPECS[name] = spec
    dvo._SUB_OPCODE_FOR_NAME[name] = opcode
    _EXP_OP = op
    return op


def _plan_tiles(n_pos: int):
    """Per-tile engine assignment.

    Returns (magic, acc_mode, groups) where acc_mode[t] is one of 'mv'
    (PE matvec), 'dve' (DVE TT into accs), 'gtt' (GPSIMD TT into acc_g),
    'grp' (CCE-DMA accumulate group), 'stt' (mixed-sign tile, DVE).
    """
    tb = n_pos // 128 if n_pos % 128 else -1
    if MV_DR:
        # mv tiles (even, <60) get fp8 exp via ACT and pair up for DR
        # matvecs, so DVE-exp (u16-bits writer) is restricted to odd tiles.
        odd = [t for t in range(5, JT - 4, 2)]
        magic = set(odd[int(round(i * (len(odd) - 1) / max(N_MAGIC - 1, 1)))]
                    for i in range(N_MAGIC))
        k = 5
        while len(magic) < N_MAGIC:
            if k % 2 and k not in magic and k < JT - 4:
                magic.add(k)
            k += 2
        acc_mode = {}
        for t in range(JT):
            if t == tb:
                acc_mode[t] = "stt"
            elif t % MV_PERIOD == 0 or t >= MV_TAIL:
                acc_mode[t] = "mv"
            else:
                acc_mode[t] = "dve"
        return magic, acc_mode, []
    # Spread DVE-exp tiles over [4, JT): the first tiles stay on ACT so the
    # pipeline fill isn't gated on the DVE finishing its startup memsets.
    if MAGIC_PAIR:
        magic = set()
        np_pairs = (N_MAGIC + 1) // 2
        for i in range(np_pairs):
            b = 4 + int(round(i * (JT - 6) / np_pairs))
            magic.add(b)
            magic.add(b + 1)
    else:
        magic = set(4 + int(round(i * (JT - 4) / N_MAGIC))
                    for i in range(N_MAGIC))
    k = 4
    while len(magic) < N_MAGIC:          # dedupe fallback
        if k not in magic:
            magic.add(k)
        k += 1

    acc_mode = {}
    for t in range(JT):
        if t == tb:
            acc_mode[t] = "stt"
        elif t % MV_PERIOD == 0 or t >= MV_TAIL or (MAGIC_MV and t in magic):
            acc_mode[t] = "mv"
        else:
            acc_mode[t] = "dve"
    if PAIR_TT:
        # Period-4 pattern [mv, dve, dve, mv]: the two dve tiles of each
        # quad land in one shared buffer and accumulate with a single
        # [128, 2048] TT — half the DVE queue slots for the same work.
        for t in range(JT):
            if t == tb or t >= MV_TAIL:
                continue
            acc_mode[t] = "dve" if t % 4 in (1, 2) else "mv"
        # Orphan a dve tile whose partner is stt/tail back to mv.
        for t in range(JT):
            if acc_mode[t] != "dve":
                continue
            partner = t + 1 if t % 4 == 1 else t - 1
            if not (0 <= partner < JT) or acc_mode[partner] != "dve":
                acc_mode[t] = "mv"

    # CCE-DMA accumulate groups: GRP_SZ sign-pure 'dve' tiles each, taken
    # from tiles < GRP_MAX so every group's accumulate DMA (each to its
    # OWN destination, so they run concurrently) lands before its fold.
    dve_tiles = [t for t in range(JT)
                 if acc_mode[t] == "dve" and t < GRP_MAX
                 and (t + 1) * 128 <= n_pos]
    groups = []
    ng = min(N_DMA_GRP, len(dve_tiles) // GRP_SZ)
    # Interleave membership so each group's slots fill slowly (one buffer
    # per group; members ~N_DMA_GRP tiles apart) and all groups finish
    # around the same, still-early tile.
    for gi in range(ng):
        g = dve_tiles[gi::ng][:GRP_SZ]
        groups.append(g)
        for t in g:
            acc_mode[t] = "grp"
    # GPSIMD TT tiles: evenly spaced from the remaining dve tiles (< 56 so
    # their ~2us TTs finish before the tail folds).
    rem = [t for t in range(JT) if acc_mode[t] == "dve" and t < 56]
    ngtt = min(N_GTT, len(rem))
    if ngtt:
        for i in range(ngtt):
            acc_mode[rem[int(round(i * (len(rem) - 1) / max(ngtt - 1, 1)))]] = "gtt"
    # Demote a few mid-loop mv tiles to DVE TT: shifts ~0.55us/tile off the
    # critical PE onto the DVE's slack.
    mv_mid = [t for t in range(8, 52)
              if acc_mode[t] == "mv" and t != tb]
    nd = min(N_DEMOTE, len(mv_mid))
    for i in range(nd):
        acc_mode[mv_mid[int(round(i * (len(mv_mid) - 1) / max(nd - 1, 1)))]] = "dve"
    return magic, acc_mode, groups


def _build_program(n_pos: int, b_is_zero: bool):
    exp_op = _register_exp_op()
    magic, acc_mode, groups = _plan_tiles(n_pos)
    grp_of = {}
    for gi, g in enumerate(groups):
        for slot, t in enumerate(g):
            grp_of[t] = (gi, slot)
    nc = bacc.Bacc()

    # fp8 DoubleRow operands: [ki, ksub, col] with d = ksub*128 + ki. xt is
    # chunk-major so each chunk DMA moves 2KB-contiguous runs per partition
    # (512B runs measured ~1/8 DMA efficiency and starved the first tiles).
    # Small duplicate "head" tensors cover the first two j-tiles and the
    # first x half so the first GEMM isn't gated on a 256KB transfer.
    xt_t = nc.dram_tensor("xt_t", [128, NCHUNK_C, 2, M // NCHUNK_C], FP8,
                          kind="ExternalInput")
    xt_h = nc.dram_tensor("xt_h", [128, 2, 256], FP8, kind="ExternalInput")
    x_t = nc.dram_tensor("x_t", [2, 128, 2, 512], FP8, kind="ExternalInput")
    cj = nc.dram_tensor("cj", [128, JT], F32, kind="ExternalInput")
    bcol = nc.dram_tensor("bcol", [128, JT], F32, kind="ExternalInput")
    sgn = nc.dram_tensor("sgn", [128, 1], BF16, kind="ExternalInput")
    w2 = nc.dram_tensor("w2", [128, 2, 32], FP8, kind="ExternalInput")
    e_q = nc.dram_tensor("e_q", [1, IC], F32, kind="ExternalInput")
    bb = nc.dram_tensor("bb", [1, 1], F32, kind="ExternalInput")
    out = nc.dram_tensor("out", [1, IC], F32, kind="ExternalOutput")
    s_out = nc.dram_tensor("s_out", [1, IC], F32, kind="ExternalOutput")

    NCHUNK = NCHUNK_C     # xt column chunks so matmuls wait on small DMAs
    CW = M // NCHUNK      # 1024 j-columns per chunk

    tb = n_pos // 128 if n_pos % 128 else -1   # mixed-sign boundary tile

    def tile_sign(t):
        return +1 if (t + 1) * 128 <= n_pos else -1

    with tile.TileContext(nc) as tc:
        with (
            tc.tile_pool(name="singles", bufs=1) as singles,
            tc.tile_pool(name="epool", bufs=EPOOL) as epool,
            tc.tile_pool(name="gpsum", bufs=3, space="PSUM") as gpsum,
            tc.tile_pool(name="spsum", bufs=1, space="PSUM") as spsum,
        ):
            # Resident inputs, all on the sync HWDGE queue (the scalar queue
            # executes on the ACT engine and steals exp throughput). Each
            # trigger costs ~600ns on the queue, so order by first use: x and
            # the first xt chunks gate the first GEMM tile; cj/nxsq gate the
            # first activation; the rest can trail.
            x_sb = [None, None]
            for ih in range(2):
                x_sb[ih] = singles.tile([128, 2, 512], FP8, tag=f"x{ih}",
                                        name=f"x{ih}")
            xth_sb = singles.tile([128, 2, 256], FP8, tag="xth")
            xt_sb = [None] * NCHUNK

            def load_chunk(ck):
                t = singles.tile([128, 2, CW], FP8, tag=f"xt_{ck}",
                                 name=f"xt{ck}")
                q = nc.scalar if (CK_SPLIT and ck % 2 == 1 and ck < 6) \
                    else nc.sync
                q.dma_start(out=t, in_=xt_t[:, ck, :, :])
                xt_sb[ck] = t

            # First-tile operands split across BOTH HWDGE queues (sync + the
            # scalar queue, whose triggers execute on the then-idle ACT
            # engine) so trigger issue doesn't serialize the pipeline fill.
            # Trigger order is by first use: GEMM operands (x halves, xt
            # head + first chunks) lead on both queues; per-tile constants
            # (cj/bcol) follow; end-of-kernel operands (e_row, b) trail.
            nc.sync.dma_start(out=x_sb[0], in_=x_t[0])
            nc.scalar.dma_start(out=xth_sb, in_=xt_h[:, :, :])
            if CK_SPLIT:
                # Third startup queue: SWDGE's ~1.7us trigger still beats
                # queueing x1 behind xth on the scalar HWDGE.
                nc.gpsimd.dma_start(out=x_sb[1], in_=x_t[1])
            else:
                nc.scalar.dma_start(out=x_sb[1], in_=x_t[1])
            load_chunk(0)
            w2_sb = singles.tile([128, 2, 32], FP8, tag="w2")
            nc.sync.dma_start(out=w2_sb, in_=w2[:, :, :])
            load_chunk(1)
            cj_sb = singles.tile([128, JT], F32, tag="cj")
            nc.scalar.dma_start(out=cj_sb, in_=cj[:, :])
            bcol_sb = singles.tile([128, JT], F32, tag="bcol")
            nc.scalar.dma_start(out=bcol_sb, in_=bcol[:, :])
            for ck in range(2, 6):
                load_chunk(ck)
            sgn_sb = singles.tile([128, 1], BF16, tag="sgn")
            nc.sync.dma_start(out=sgn_sb, in_=sgn[:, :])
            for ck in range(6, NCHUNK):
                load_chunk(ck)
            e_row = singles.tile([1, IC], F32, tag="e_row")
            nc.sync.dma_start(out=e_row, in_=e_q[:, :])
            b_sb = singles.tile([1, 1], F32, tag="b")
            nc.sync.dma_start(out=b_sb, in_=bb[:, :])

            # Warm the PE while input DMAs are in flight (HAM clock gate).
            # warm_w memset rides GPSIMD (which starts earliest) so warmups
            # aren't gated on the DVE's slower queue startup.
            warm_w = singles.tile([128, 128], BF16, tag="warm_w")
            nc.gpsimd.memset(warm_w, 0.0)
            warm_ps = spsum.tile([1, 128], F32, tag="s")
            for _ in range(N_WARM):
                nc.tensor.matmul(
                    out=warm_ps, lhsT=warm_w[:, 0:1], rhs=warm_w[:, :],
                    start=True, stop=True,
                )
            # Dummy activation with no DMA dependency: pulls the ~1.3us exp
            # table load off the first real tile's critical path.
            twarm = singles.tile([1, 8], F32, tag="twarm")
            nc.scalar.activation(
                out=twarm, in_=warm_w[0:1, 0:8],
                func=mybir.ActivationFunctionType.Exp,
            )

            ones_sb = singles.tile([128, 1], BF16, tag="ones")
            nc.vector.memset(ones_sb, 1.0)
            mones_sb = singles.tile([128, 1], BF16, tag="mones")
            nc.vector.memset(mones_sb, -1.0)
            accs = singles.tile([128, IC], BF16, tag="accs")
            nc.vector.memset(accs, 0.0)
            acc2 = None
            if PAIR_TT:
                acc2 = singles.tile([128, 2 * IC], BF16, tag="acc2")
                nc.vector.memset(acc2, 0.0)
            acc_g = None
            if any(m == "gtt" for m in acc_mode.values()):
                acc_g = singles.tile([128, IC], BF16, tag="acc_g")
                nc.gpsimd.memset(acc_g, 0.0)
            acc_d = []
            for gi in range(len(groups)):
                a = singles.tile([128, IC], BF16, tag=f"acc_d{gi}")
                nc.vector.memset(a, 0.0)
                acc_d.append(a)


            # s_ps accumulates PE-matvec'd tiles across the whole j-loop and
            # receives the folds of acc2/accs at the end. Shares the "s"
            # PSUM slot with warm_ps; the first matvec's start=True clears it.
            s_ps = spsum.tile([1, IC], F32, tag="s")

            e_views = {}          # t -> bf16-view AP of tile t's exp output
            e4_tiles = {}         # group idx -> wide [128, 4, IC] tile
            pair_bufs = {}        # quad idx -> shared [128, 2*IC] tile
            mvdr_bufs = {}        # mv-pair idx -> fp8 [128, 2, IC] tile
            first_mv = [True]

            def pair_quad(t):
                return (t - 1) // 4 if t % 4 == 1 else (t - 2) // 4

            def mvdr_paired(t):
                # mv pairs (4k, 4k+2), both below the bf16 tail, same sign
                return (MV_DR == 1 and acc_mode.get(t) == "mv" and t < MV_TAIL
                        and t % 2 == 0 and (t | 2) < MV_TAIL
                        and acc_mode.get(t ^ 2) == "mv"
                        and tile_sign(t) == tile_sign(t ^ 2))

            PM = (mybir.MatmulPerfMode.DoubleRowSwInterleave if SWI
                  else mybir.MatmulPerfMode.DoubleRow)

            def emit_gemm(t):
                if SWI:
                    # Interleaved weights: tile t's 256 fp8 are contiguous
                    # per partition at flat offset (t%4)*256 in its chunk.
                    if t < 2:
                        lhsT = xth_sb[:, t, :]
                    else:
                        fl = (t % 4) * 256
                        lhsT = xt_sb[t // 4][:, fl // CW, fl % CW:fl % CW + 256]
                elif t < 2:
                    lhsT = xth_sb[:, :, t * 128:(t + 1) * 128]
                else:
                    ck, col = (t * 128) // CW, (t * 128) % CW
                    lhsT = xt_sb[ck][:, :, col:col + 128]
                g_ps = gpsum.tile([128, IC], F32, tag="g", name=f"g{t}")
                for ic in range(2):
                    nc.tensor.matmul(
                        out=g_ps[:, ic * 512:(ic + 1) * 512],
                        lhsT=lhsT,
                        rhs=x_sb[ic][:, :, :],
                        start=True, stop=True,
                        perf_mode=PM,
                    )
                return g_ps

            def emit_exp(t, g_ps):
                if t in grp_of:
                    gi, slot = grp_of[t]
                    if gi not in e4_tiles:
                        # Private buffer per group: pool-slot recycling under
                        # an in-flight accumulate DMA read is both a stall
                        # and a tracking hazard.
                        e4_tiles[gi] = singles.tile(
                            [128, GRP_SZ, IC], BF16, tag=f"e4_{gi}",
                            name=f"e4_{gi}")
                    dest = e4_tiles[gi][:, slot, :]
                elif PAIR_TT and acc_mode[t] == "dve":
                    q = pair_quad(t)
                    if q not in pair_bufs:
                        pair_bufs[q] = singles.tile(
                            [128, 2 * IC], BF16, tag=f"pb{q}", name=f"pb{q}")
                    slot = 0 if t % 4 == 1 else 1
                    dest = pair_bufs[q][:, slot * IC:(slot + 1) * IC]
                elif mvdr_paired(t):
                    # fp8 exp output pair, consumed by one DR matvec
                    q = t // 4
                    if q not in mvdr_bufs:
                        mvdr_bufs[q] = singles.tile(
                            [128, 2, IC], FP8, tag=f"mp{q}", name=f"mp{q}")
                    dest = mvdr_bufs[q][:, (t >> 1) & 1, :]
                else:
                    # One private buffer per tile (no pool-slot reuse): the
                    # slot-free dependency would cost a separate ~100ns sem
                    # instruction on the producing engine's queue.
                    dest = singles.tile([128, IC], BF16, tag=f"e{t}",
                                        name=f"e{t}")
                if t in magic:
                    # Single full-tile drain: splitting this into halves to
                    # exploit subtile deps was measured ~4us slower (extra
                    # DVE queue ops outweigh the earlier PSUM-slot release).
                    nc.vector._custom_dve(
                        exp_op, out=dest.bitcast(U16), in0=g_ps,
                        s0=SCH_A, s1=bcol_sb[:, t:t + 1], imm2=SCH_CLAMP,
                    )
                else:
                    nc.scalar.activation(
                        out=dest, in_=g_ps,
                        func=mybir.ActivationFunctionType.Exp,
                        bias=cj_sb[:, t:t + 1], scale=1.0,
                    )
                e_views[t] = dest

            def emit_fold(f, w, stop=False):
                if isinstance(f, str) and f == "ACC2":
                    # acc2 holds two interleaved [128, IC] partial sums.
                    for k in range(2):
                        for ic in range(2):
                            sl = slice(ic * 512, (ic + 1) * 512)
                            nc.tensor.matmul(
                                out=s_ps[:, sl], lhsT=ones_sb,
                                rhs=acc2[:, k * IC + ic * 512:
                                         k * IC + (ic + 1) * 512],
                                start=first_mv[0], stop=False,
                                skip_group_check=True,
                            )
                            first_mv[0] = False
                    return
                if MV_ONE:
                    nc.tensor.matmul(
                        out=s_ps[:, :], lhsT=w, rhs=f[:, :],
                        start=first_mv[0], stop=stop,
                        skip_group_check=True,
                    )
                    first_mv[0] = False
                    return
                for ic in range(2):
                    sl = slice(ic * 512, (ic + 1) * 512)
                    nc.tensor.matmul(
                        out=s_ps[:, sl], lhsT=w, rhs=f[:, sl],
                        start=first_mv[0], stop=stop,
                        skip_group_check=True,
                    )
                    first_mv[0] = False

            def emit_acc(t, last=False):
                mode = acc_mode[t]
                e_t = e_views.pop(t)
                if mode == "dve":
                    if PAIR_TT:
                        if t % 4 == 1:
                            return   # partner's acc handles the pair
                        pb = pair_bufs.pop(pair_quad(t))
                        if tile_sign(t) > 0:
                            nc.vector.tensor_add(acc2, acc2, pb)
                        else:
                            nc.vector.tensor_sub(acc2, acc2, pb)
                    elif tile_sign(t) > 0:
                        nc.vector.tensor_add(accs, accs, e_t)
                    else:
                        nc.vector.tensor_sub(accs, accs, e_t)
                elif mode == "gtt":
                    nc.gpsimd.tensor_add(acc_g, acc_g, e_t)
                elif mode == "grp":
                    gi, slot = grp_of[t]
                    if slot != GRP_SZ - 1:
                        return     # one CCE DMA per completed group
                    e4 = e4_tiles.pop(gi)
                    nc.gpsimd.dma_start(
                        out=acc_d[gi].unsqueeze(1).broadcast_to(
                            [128, GRP_SZ, IC]),
                        in_=e4,
                        accum_op=mybir.AluOpType.add,
                    )
                elif mode == "stt":
                    nc.vector.scalar_tensor_tensor(
                        out=accs, in0=e_t, scalar=sgn_sb[:, 0:1], in1=accs,
                        op0=mybir.AluOpType.mult, op1=mybir.AluOpType.add,
                    )
                elif mvdr_paired(t):
                    if t % 4 == 0:
                        return      # partner (t+2) emits the pair's matvec
                    ep = mvdr_bufs.pop(t // 4)
                    wsl = w2_sb[:, :, 0:1] if tile_sign(t) > 0 \
                        else w2_sb[:, :, 16:17]
                    for ic in range(2):
                        sl = slice(ic * 512, (ic + 1) * 512)
                        nc.tensor.matmul(
                            out=s_ps[:, sl], lhsT=wsl, rhs=ep[:, :, sl],
                            start=first_mv[0], stop=False,
                            perf_mode=mybir.MatmulPerfMode.DoubleRow,
                            skip_group_check=True,
                        )
                    first_mv[0] = False
                else:
                    w = ones_sb if tile_sign(t) > 0 else mones_sb
                    if MV_ONE:
                        nc.tensor.matmul(
                            out=s_ps[:, :], lhsT=w, rhs=e_t[:, :],
                            start=first_mv[0], stop=last,
                            skip_group_check=True,
                        )
                    else:
                        for ic in range(2):
                            sl = slice(ic * 512, (ic + 1) * 512)
                            nc.tensor.matmul(
                                out=s_ps[:, sl], lhsT=w, rhs=e_t[:, sl],
                                start=first_mv[0], stop=last,
                                skip_group_check=True,
                            )
                    first_mv[0] = False

            # Accumulator folds are emitted inline, each some tiles after
            # its accumulator's last contribution (for the DMA groups, late
            # enough that the ~1.5MB accumulate DMA has landed), so the tail
            # after the last GEMM is just exp + final matvec + p_row + out.
            last_contrib = {"accs": -1, "acc2": -1, "acc_g": -1}
            for t in range(JT):
                m = acc_mode[t]
                if m == "stt" or (m == "dve" and not PAIR_TT):
                    last_contrib["accs"] = max(last_contrib["accs"], t)
                elif m == "dve":
                    last_contrib["acc2"] = max(last_contrib["acc2"], t)
                elif m == "gtt":
                    last_contrib["acc_g"] = max(last_contrib["acc_g"], t)
            fold_at = {}       # loop index -> list of (acc, weight)
            GRP_LAG, TT_LAG = _env("KN_GRPLAG", 14), 3
            for gi, g in enumerate(groups):
                w = ones_sb if tile_sign(g[0]) > 0 else mones_sb
                fold_at.setdefault(
                    min(g[-1] + GRP_LAG, JT - 2), []).append((acc_d[gi], w))
            if last_contrib["accs"] >= 0:
                fold_at.setdefault(
                    min(last_contrib["accs"] + TT_LAG, JT - 2), []).append(
                    (accs, ones_sb))
            if last_contrib["acc2"] >= 0:
                fold_at.setdefault(
                    min(last_contrib["acc2"] + TT_LAG, JT - 2), []).append(
                    ("ACC2", None))
            if last_contrib["acc_g"] >= 0:
                # GPSIMD TTs run ~2us and may queue behind a group-DMA
                # trigger; give this fold extra slack.
                fold_at.setdefault(
                    min(last_contrib["acc_g"] + 2 * TT_LAG, JT - 2), []).append(
                    (acc_g, ones_sb))

            last_mv = max(t for t in range(JT) if acc_mode[t] == "mv")
            for t in range(JT):
                g_ps = emit_gemm(t)
                emit_exp(t, g_ps)
                if t >= ACC_LAG:
                    ta = t - ACC_LAG
                    emit_acc(ta, last=ta == last_mv)
                    for f, w in fold_at.get(ta, ()):
                        emit_fold(f, w)
            for ta in range(JT - ACC_LAG, JT):
                emit_acc(ta, last=ta == last_mv)
                for f, w in fold_at.get(ta, ()):
                    emit_fold(f, w)

            # The PSUM accumulation chain stopped on the last matvec; now
            # scale by the query-side factor and ship out, half by half so
            # half 0's DMA overlaps half 1's multiply.
            p_row = singles.tile([1, IC], F32, tag="p_row")
            for ic in range(2):
                sl = slice(ic * 512, (ic + 1) * 512)
                nc.vector.tensor_mul(p_row[:, sl], s_ps[:, sl], e_row[:, sl])
                if not b_is_zero:
                    nc.vector.tensor_scalar(
                        out=p_row[:, sl], in0=p_row[:, sl],
                        scalar1=b_sb[0:1, 0:1], scalar2=None,
                        op0=mybir.AluOpType.add,
                    )
                q = nc.scalar if ic == 0 else nc.sync
                q.dma_start(out=out[:, sl], in_=p_row[:, sl])
            if DEBUG_S:
                s_sb = singles.tile([1, IC], F32, tag="s_sb")
                nc.vector.tensor_copy(s_sb, s_ps)
                nc.sync.dma_start(out=s_out[:, :], in_=s_sb)  # pre-mask s

    nc.finalize()
    return nc


def kernel(X, X_train, alphas, y_train, b):
    X = np.ascontiguousarray(np.asarray(X, dtype=np.float32))
    X_train = np.ascontiguousarray(np.asarray(X_train, dtype=np.float32))
    alphas = np.asarray(alphas, dtype=np.float32).reshape(M)
    y_train = np.asarray(y_train, dtype=np.float32).reshape(M)
    b_arr = np.asarray(b, dtype=np.float32).reshape(1, 1)

    # Sort train points by label (+1 first), then by c within each label so
    # c values on a partition row are close (enables shared-bias tricks).
    c_all = (-GAMMA * (X_train * X_train).sum(1)
             + np.log(np.maximum(alphas, np.float32(1e-38)))).astype(np.float32)
    perm = np.lexsort((c_all, -y_train))
    n_pos = int((y_train > 0).sum())
    Xt_p = X_train[perm]
    c = c_all[perm]

    # Global exponent shift M = max(G + c): folded into the cj bias so all
    # exp outputs are <= 1 (mv-pair tiles then fit fp8e4 range; fp8's 448
    # max leaves ~6 e-folds of slack over the quantization-error margin).
    # Compensated exactly in the final per-row scale e_q * e^M.
    global LAST_M
    msh = 0.0
    if MV_DR:
        colmax = (X @ X_train.T).max(axis=0)     # true per-train-point max G
        msh = float((colmax + c_all).max()) + 0.5  # +fp8-quantization margin
    LAST_M = msh

    cj = np.ascontiguousarray(c.reshape(JT, 128).T) - np.float32(msh)
    bcol = (SCH_A * cj + np.float32(SCH_B)).astype(np.float32)
    w2_arr = np.zeros((128, 2, 32), dtype=ml_dtypes.float8_e4m3fn)
    w2_arr[:, :, 0] = 1.0
    w2_arr[:, :, 16] = -1.0
    r = n_pos % 128
    sgn_vec = np.where(np.arange(128) < r, 1.0, -1.0).astype(
        ml_dtypes.bfloat16).reshape(128, 1)

    # fp8 DoubleRow layouts: [ki, ksub, col], d = ksub*128 + ki; xt stored
    # chunk-major: [ki, chunk, ksub, j_local].
    f8 = ml_dtypes.float8_e4m3fn
    cw = M // NCHUNK_C
    if SWI:
        # DoubleRowSwInterleave: per partition, each 128-col weight block is
        # stored as [A127, B127, A126, B126, ..., A0, B0] (A=ksub0, B=ksub1,
        # columns reversed) — contiguous 256B per partition per tile.
        A = Xt_p.T.reshape(2, 128, M)                       # [ksub, ki, j]
        B = A.reshape(2, 128, JT, 128)[:, :, :, ::-1]       # [ksub, ki, t, c']
        sw = np.ascontiguousarray(
            B.transpose(1, 2, 3, 0)).reshape(128, JT, 256)  # [ki, t, 2c'+i]
        xt_dr = np.ascontiguousarray(
            sw.reshape(128, NCHUNK_C, 2, cw).astype(f8))
        xt_h = np.ascontiguousarray(sw[:, 0:2, :].astype(f8))
    else:
        xt_dr = np.ascontiguousarray(
            Xt_p.T.reshape(2, 128, NCHUNK_C, cw).transpose(1, 2, 0, 3).astype(f8))
    # Query-side factor exp(-g*||x||^2) with fp32-FTZ emulation (the
    # reference's direct exp(-g*d) underflows to exact 0); host-computed
    # O(N*D) prep like the c_j constants.
    e_full = np.exp((-GAMMA * (X * X).sum(1) + msh).astype(np.float32))
    e_full = np.where(e_full >= np.float32(1.1754944e-38), e_full,
                      np.float32(0.0)).astype(np.float32)

    if not SWI:
        xt_h = np.ascontiguousarray(
            Xt_p[0:256].T.reshape(2, 128, 256).transpose(1, 0, 2).astype(f8))
    in_maps = []
    for k in range(NCORES):
        sl = slice(k * IC, (k + 1) * IC)
        # [ihalf, ki, ksub, i_local]
        x_dr = np.ascontiguousarray(
            X[sl].T.reshape(2, 128, IC).transpose(1, 0, 2)
            .reshape(128, 2, 2, 512).transpose(2, 0, 1, 3).astype(f8))
        in_maps.append({
            "xt_t": xt_dr,
            "xt_h": xt_h,
            "x_t": x_dr,
            "cj": cj,
            "bcol": bcol,
            "sgn": sgn_vec,
            "w2": w2_arr,
            "e_q": np.ascontiguousarray(e_full[sl].reshape(1, IC)),
            "bb": b_arr,
        })

    nc = _build_program(n_pos, b_is_zero=float(b_arr.reshape(-1)[0]) == 0.0)
    res = run_bass_kernel_spmd(nc, in_maps, list(range(NCORES)), trace=TRACE)
    global LAST_RESULTS
    LAST_RESULTS = res

    preds = np.concatenate([res.results[k]["out"][0] for k in range(NCORES)])
    return preds.reshape(N, 1).astype(np.float32)

